# revision 13
# baseline (speedup 1.0000x reference)
"""Trainium2 Bass kernel for nn_MetricLoss (segment_reduce / discriminative loss).

Reference math (K=32 labels, D=16):
  cents[s,k,:]  = mean of embeddings of sample s where label==k
  push[s]       = sum_{k<j} relu(0.25 - L1(c_sk, c_sj))^2 / 496
  pull[s]       = mean over ALL B*H*W pixels p of  L1(e_p, c_s,label_p)^2
  loss          = mean_s (push[s] + 0.1 * pull[s])

Key idea: HOST-SORT each core's 73728 pixels by label (order-invariant for
both the segment sums and the pull mean).  Sorted runs are padded to whole
128-pixel tiles (TCB=608 tiles/core, provably enough) so every tile has a
single label.  This removes the one-hot from both launches:

  Launch A (per-tile column sums):
    - embA fp8 [128, 76 groups * 8 tiles * 16 d]; per group g one matmul
      lhsT = embA[:, g] (128 pix x 128 cols), rhs = ones [128, 1] ->
      psum[:, g] = per-(tile,dim) sums over the 128 pixels.  psum [128, 76]
      f32 -> tiny evac + DMA.  Host groups tiles by label -> cents (f64),
      push term on host.

  Launch B (pull term):
    - per tile ONE matmul, contraction rows = 17:
        lhsT (weights) = [embT (16 rows, fp8) ; mask/64 (1 row)]  [17, 128]
        rhs (moving)   = [-I_16 per b (16 rows) ; 64*c_b,lab(t),d (1 row)]
      psum out [pix, (b,d)] = c_b,lab(t),d - e_pix,d  (0 for pad pixels).
    - weights+rhs fused in one fp8 HBM tensor 'wrt' [2 rowgrp, 17, 304, 192]
      (even tiles use PE rows 0:17, odd rows 64:81 -> LDW/MM overlap).
    - 19 psum waves of 32 tiles ([128, 32, 4, 16] f32 = 4 banks, ping-pong).
      Consumer pattern per wave balances the three PSUM/SBUF engines:
        'D' = DVE tensor_reduce(|.|, sum over d) straight from psum
        'A' = ACT Abs -> bf16, Pool h1 halving, DVE h2 + reduce
        'P' = ACT Abs -> bf16, Pool h1+h2+h3 halvings, DVE final halving
    - dist [128, 19, 32, 4] bf16 -> tensor_tensor_reduce (dist^2, accum)
      -> pacc [128, 2] f32 -> host mean.
"""

import os

import numpy as np
import ml_dtypes

import concourse.bass as bass
import concourse.bacc as bacc
import concourse.mybir as mybir
from concourse.tile import TileContext
from concourse.bass_utils import run_bass_kernel_spmd

BF16 = ml_dtypes.bfloat16
FP8 = ml_dtypes.float8_e4m3
F32 = np.float32

# problem constants (hardcoded per contract)
B, H, W, D, K = 4, 384, 384, 16, 32
NCORES = 8
NPIX_TOT = B * H * W              # 589824
NPIX = NPIX_TOT // NCORES         # 73728 per core
P = 128                           # partitions
TCB = 608                         # padded tiles/core: sum_k ceil(cnt_k/128) <= 607
NG_A = TCB // 8                   # 76 matmul groups in launch A
NWAVE = TCB // 32                 # 19 psum waves in launch B
TH = TCB // 2                     # 304 tile-halves per row group

PUSH_MARGIN = 0.25
PUSH_W = 1.0
PULL_W = 0.1
NCMP = K * (K - 1) / 2.0

# per-wave consumer kinds for launch B (len NWAVE = 19)
#   D: DVE abs-reduce from psum; A: ACT abs + Pool h1 + DVE h2+reduce;
#   P: ACT abs + Pool h1+h2+h3 + DVE final halving
CONSUMER_PATTERN = os.environ.get("DBG_PATTERN", "DPADPADPADPDAPDPDAD")
USE_TTR = os.environ.get("DBG_TTR", "0") == "1"
# contraction rows: 16 embT + 1 mask row.  NOTE: even/odd tiles run in PE row
# groups 0/2 concurrently; their psum writes MUST land in different banks
# (slot = rg*16 + j//2 below) or the PE hangs (row-tile PSUM bank conflict).
CR = int(os.environ.get("DBG_CR", "17"))

_built = {}


# --------------------------------------------------------------------------
# host-side sort / tiling
# --------------------------------------------------------------------------

def _sort_core(lab):
    """lab [NPIX] int32 -> (order, tile_lab, cnt).

    order: pixel permutation grouping labels; tile_lab [TCB]: label of each
    128-pixel tile (runs padded to whole tiles; leftover tiles labeled 0 and
    fully zero-padded).
    """
    order = np.argsort(lab, kind="stable")
    cnt = np.bincount(lab, minlength=K)
    tile_lab = np.zeros(TCB, dtype=np.int64)
    t = 0
    for k in range(K):
        nt = -(-int(cnt[k]) // P)
        tile_lab[t : t + nt] = k
        t += nt
    return order, tile_lab, cnt, t


def _tiled_emb(emb, lab):
    """emb [NPIX, D] f32, lab [NPIX] -> (tiles [TCB, P, D] f32,
    mask [TCB, P] f32, tile_lab [TCB], cnt [K])."""
    order, tile_lab, cnt, ntl = _sort_core(lab)
    es = emb[order]
    tiles = np.zeros((TCB, P, D), dtype=F32)
    mask = np.zeros((TCB, P), dtype=F32)
    off = 0
    t = 0
    for k in range(K):
        n = int(cnt[k])
        nt = -(-n // P)
        buf = np.zeros((nt * P, D), dtype=F32)
        buf[:n] = es[off : off + n]
        tiles[t : t + nt] = buf.reshape(nt, P, D)
        mbuf = np.zeros(nt * P, dtype=F32)
        mbuf[:n] = 1.0
        mask[t : t + nt] = mbuf.reshape(nt, P)
        off += n
        t += nt
    return tiles, mask, tile_lab, cnt


# --------------------------------------------------------------------------
# launch A: per-tile column sums
# --------------------------------------------------------------------------

def _build_launch_a():
    nc = bacc.Bacc("TRN2", target_bir_lowering=False, debug=False)
    f8 = mybir.dt.float8e4
    bf = mybir.dt.bfloat16
    f32 = mybir.dt.float32

    embA = nc.dram_tensor("embA", [P, TCB * D], f8, kind="ExternalInput")
    outA = nc.dram_tensor("outA", [P, NG_A], f32, kind="ExternalOutput")

    with TileContext(nc) as tc:
        with (
            tc.tile_pool(name="sbuf", bufs=1) as pool,
            tc.tile_pool(name="psum", bufs=1, space="PSUM") as psum_pool,
        ):
            emb_sb = pool.tile([P, TCB * D], f8)
            ones = pool.tile([P, 1], bf)
            evac = pool.tile([P, NG_A], f32)
            psA = psum_pool.tile([P, NG_A], f32)

            nc.vector.memset(ones[:], 1.0)
            NCH = 2
            ch = (TCB * D) // NCH
            for i in range(NCH):
                nc.sync.dma_start(
                    out=emb_sb[:, i * ch : (i + 1) * ch],
                    in_=embA.ap()[:, i * ch : (i + 1) * ch],
                )
            for g in range(NG_A):
                nc.tensor.matmul(
                    psA[:, g : g + 1],
                    emb_sb[:, 128 * g : 128 * g + 128],
                    ones[:],
                    start=True,
                    stop=True,
                )
            nc.vector.tensor_copy(out=evac[:], in_=psA[:])
            nc.sync.dma_start(out=outA.ap(), in_=evac[:])
    nc.compile()
    return nc


def _prep_a(emb_flat, lab_flat):
    in_maps = []
    for c in range(NCORES):
        emb = emb_flat[c * NPIX : (c + 1) * NPIX]
        lab = lab_flat[c * NPIX : (c + 1) * NPIX]
        tiles, _, _, _ = _tiled_emb(emb, lab)
        # embA[p, g, j, d] = tiles[8g + j, p, d]
        embA = np.ascontiguousarray(
            tiles.reshape(NG_A, 8, P, D).transpose(2, 0, 1, 3)
        ).astype(FP8)
        in_maps.append({"embA": embA.reshape(P, TCB * D)})
    return in_maps


def _reduce_a(results, lab_flat):
    """outA [8][P, NG_A] -> cents [B, K, D] float64, counts [B, K]."""
    sums = np.zeros((B, K, D), dtype=np.float64)
    for c in range(NCORES):
        lab = lab_flat[c * NPIX : (c + 1) * NPIX]
        _, tile_lab, _, _ = _sort_core(lab)
        o = results[c]["outA"].astype(np.float64)  # [P=(j,d), g]
        ts = o.reshape(8, D, NG_A).transpose(2, 0, 1).reshape(TCB, D)  # [t, d]
        np.add.at(sums[c // 2], tile_lab, ts)
    cnts = np.zeros((B, K), dtype=np.int64)
    spl = NPIX_TOT // B
    for b in range(B):
        cnts[b] = np.bincount(lab_flat[b * spl : (b + 1) * spl], minlength=K)
    cents = sums / np.maximum(cnts, 1)[:, :, None]
    cents = np.where(cnts[:, :, None] > 0, cents, 0.0)
    return cents, cnts


# --------------------------------------------------------------------------
# launch B: pull term
# --------------------------------------------------------------------------

def _build_launch_b():
    nc = bacc.Bacc("TRN2", target_bir_lowering=False, debug=False)
    f8 = mybir.dt.float8e4
    bf = mybir.dt.bfloat16
    f32 = mybir.dt.float32
    AF = mybir.ActivationFunctionType

    # per row group rg: [CR rows, TH tile-halves, 192 cols (128 w + 64 rhs)]
    wrt = nc.dram_tensor("wrt", [2 * CR, TH * 192], f8, kind="ExternalInput")
    pacc_d = nc.dram_tensor("pacc", [P, 2], f32, kind="ExternalOutput")

    with TileContext(nc) as tc:
        with (
            tc.tile_pool(name="sbuf", bufs=1) as pool,
            tc.tile_pool(name="work", bufs=3) as wpool,
            tc.tile_pool(name="psum", bufs=2, space="PSUM") as psum_pool,
        ):
            wrt_sb = pool.tile([P, TH, 192], f8)   # rows 0:17 even, 64:81 odd
            dist = pool.tile([P, NWAVE, 32, 4], bf)
            sqb = pool.tile([P, NWAVE * 32 * 4], bf)
            pacc = pool.tile([P, 2], f32)

            # interleave row-group chunks so wave 0's tiles arrive first
            bounds = [0, 48, 176, TH]
            for i in range(len(bounds) - 1):
                sl = slice(bounds[i], bounds[i + 1])
                for rg in range(2):
                    nc.sync.dma_start(
                        out=wrt_sb[64 * rg : 64 * rg + CR, sl, :].rearrange(
                            "r t m -> r (t m)"
                        ),
                        in_=wrt.ap()[
                            CR * rg : CR * rg + CR,
                            bounds[i] * 192 : bounds[i + 1] * 192,
                        ],
                    )

            for w in range(NWAVE):
                ps = psum_pool.tile([P, 32, 4, D], f32, tag="ps", name=f"ps_{w}")
                for j in range(32):
                    t = 32 * w + j
                    rg, th = t % 2, t // 2
                    slot = rg * 16 + j // 2   # rg0 -> banks 0-1, rg1 -> 2-3
                    nc.tensor.matmul(
                        ps[:, slot, :, :].rearrange("p a b -> p (a b)"),
                        wrt_sb[64 * rg : 64 * rg + CR, th, 0:128],
                        wrt_sb[64 * rg : 64 * rg + CR, th, 128:192],
                        start=True,
                        stop=True,
                    )
                kind = CONSUMER_PATTERN[w]
                with nc.allow_low_precision("dist bf16; error averages out"):
                    if kind == "D":
                        nc.vector.tensor_reduce(
                            out=dist[:, w, :, :],
                            in_=ps[:],
                            axis=mybir.AxisListType.X,
                            op=mybir.AluOpType.add,
                            apply_absolute_value=True,
                        )
                    else:
                        a4 = wpool.tile([P, 32, 4, D], bf, tag="a4")
                        h1 = wpool.tile([P, 32, 4, D // 2], bf, tag="h1")
                        nc.scalar.activation(
                            out=a4[:].rearrange("p a b c -> p (a b c)"),
                            in_=ps[:].rearrange("p a b c -> p (a b c)"),
                            func=AF.Abs,
                        )
                        nc.gpsimd.tensor_tensor(
                            out=h1[:],
                            in0=a4[:, :, :, 0 : D // 2],
                            in1=a4[:, :, :, D // 2 : D],
                            op=mybir.AluOpType.add,
                        )
                        if kind == "A":
                            h2 = wpool.tile([P, 32, 4, D // 4], bf, tag="h2")
                            nc.vector.tensor_tensor(
                                out=h2[:],
                                in0=h1[:, :, :, 0 : D // 4],
                                in1=h1[:, :, :, D // 4 : D // 2],
                                op=mybir.AluOpType.add,
                            )
                            nc.vector.tensor_reduce(
                                out=dist[:, w, :, :],
                                in_=h2[:],
                                axis=mybir.AxisListType.X,
                                op=mybir.AluOpType.add,
                            )
                        else:  # P: pool does h2+h3, DVE final halving add
                            h2 = wpool.tile([P, 32, 4, D // 4], bf, tag="h2")
                            h3 = wpool.tile([P, 32, 4, D // 8], bf, tag="h3")
                            nc.gpsimd.tensor_tensor(
                                out=h2[:],
                                in0=h1[:, :, :, 0 : D // 4],
                                in1=h1[:, :, :, D // 4 : D // 2],
                                op=mybir.AluOpType.add,
                            )
                            nc.gpsimd.tensor_tensor(
                                out=h3[:],
                                in0=h2[:, :, :, 0:2],
                                in1=h2[:, :, :, 2:4],
                                op=mybir.AluOpType.add,
                            )
                            nc.vector.tensor_tensor(
                                out=dist[:, w, :, :],
                                in0=h3[:, :, :, 0],
                                in1=h3[:, :, :, 1],
                                op=mybir.AluOpType.add,
                            )

            # pull accumulation: dist^2 summed, f32 accumulators
            flat = dist[:].rearrange("p a b c -> p (a b c)")
            NTOT = NWAVE * 32 * 4
            hsp = (NWAVE // 2) * 32 * 4
            with nc.allow_low_precision("sq bf16; accum f32"):
                for i, sl in enumerate(
                    (slice(0, hsp), slice(hsp, NTOT))
                ):
                    if USE_TTR:
                        nc.vector.tensor_tensor_reduce(
                            out=sqb[:, sl],
                            in0=flat[:, sl],
                            in1=flat[:, sl],
                            scale=1.0,
                            scalar=0.0,
                            op0=mybir.AluOpType.mult,
                            op1=mybir.AluOpType.add,
                            accum_out=pacc[:, i : i + 1],
                        )
                    else:
                        nc.vector.tensor_tensor(
                            out=sqb[:, sl],
                            in0=flat[:, sl],
                            in1=flat[:, sl],
                            op=mybir.AluOpType.mult,
                        )
                        nc.vector.tensor_reduce(
                            out=pacc[:, i : i + 1],
                            in_=sqb[:, sl],
                            axis=mybir.AxisListType.X,
                            op=mybir.AluOpType.add,
                        )
            nc.sync.dma_start(out=pacc_d.ap(), in_=pacc[:])
    nc.compile()
    return nc


def _prep_b(emb_flat, lab_flat, cents):
    """Fused weights+rhs table 'wrt' per core."""
    cb = cents.astype(F32)  # [B, K, D]
    neg_eye = np.tile(-np.eye(D, dtype=F32), (1, 4))  # [16, 64]
    in_maps = []
    for c in range(NCORES):
        emb = emb_flat[c * NPIX : (c + 1) * NPIX]
        lab = lab_flat[c * NPIX : (c + 1) * NPIX]
        tiles, mask, tile_lab, _ = _tiled_emb(emb, lab)
        # weights [TCB, CR, 128] (rows 17:CR zero-padded)
        w17 = np.zeros((TCB, CR, P), dtype=F32)
        w17[:, 0:D, :] = tiles.transpose(0, 2, 1)
        w17[:, D, :] = mask / 64.0
        # rhs [TCB, CR, 64]
        r17 = np.zeros((TCB, CR, 4 * D), dtype=F32)
        r17[:, 0:D, :] = neg_eye[None]
        crow = cb[:, tile_lab, :].transpose(1, 0, 2).reshape(TCB, 4 * D)
        r17[:, D, :] = 64.0 * crow
        fused = np.concatenate([w17, r17], axis=2)  # [TCB, CR, 192]
        wrt = np.zeros((2, CR, TH, 192), dtype=FP8)
        wrt[0] = fused[0::2].transpose(1, 0, 2).astype(FP8)
        wrt[1] = fused[1::2].transpose(1, 0, 2).astype(FP8)
        in_maps.append({"wrt": wrt.reshape(2 * CR, TH * 192)})
    return in_maps


def _push_host(cents):
    """push[b] = sum_{k<j} relu(m - L1(c_k, c_j))^2 / 496 on host (tiny)."""
    cb = cents.astype(np.float64)  # [B, K, D]
    d = np.abs(cb[:, :, None, :] - cb[:, None, :, :]).sum(axis=-1)  # [B, K, K]
    m = np.maximum(PUSH_MARGIN - d, 0.0)
    iu = np.triu(np.ones((K, K), dtype=bool), k=1)
    return (m * m * iu[None]).sum(axis=(1, 2)) / NCMP  # [B]


def _get(name):
    if name not in _built:
        if name == "A":
            _built[name] = _build_launch_a()
        else:
            _built[name] = _build_launch_b()
    return _built[name]


def run_launches(embeddings, labels, trace=False, trace_kwargs=None):
    """Returns (loss_scalar, resA, resB) — resA/resB are BassKernelResults."""
    emb_flat = np.ascontiguousarray(np.asarray(embeddings), dtype=F32).reshape(
        NPIX_TOT, D
    )
    lab_flat = np.ascontiguousarray(np.asarray(labels), dtype=np.int32).reshape(
        NPIX_TOT
    )
    core_ids = list(range(NCORES))

    kwA = dict(trace=trace, **(trace_kwargs or {}))
    resA = run_bass_kernel_spmd(_get("A"), _prep_a(emb_flat, lab_flat), core_ids, **kwA)
    cents, _ = _reduce_a(resA.results, lab_flat)

    resB = run_bass_kernel_spmd(
        _get("B"), _prep_b(emb_flat, lab_flat, cents), core_ids, **kwA
    )
    pull_sum = 0.0
    for c in range(NCORES):
        pull_sum += float(resB.results[c]["pacc"].astype(np.float64).sum())
    pull_mean = pull_sum / NPIX_TOT / B  # mean over b of pull[b]

    push = _push_host(cents)
    loss = np.mean(PUSH_W * push) + PULL_W * pull_mean
    return np.array(loss, dtype=F32), resA, resB


def kernel(embeddings, labels):
    loss, _, _ = run_launches(embeddings, labels, trace=False)
    return loss


# revision 16
# speedup vs baseline: 1.4186x; 1.4186x over previous
"""Trainium2 Bass kernel for nn_MetricLoss (segment_reduce / discriminative loss).

Reference math (K=32 labels, D=16):
  cents[s,k,:]  = mean of embeddings of sample s where label==k
  push[s]       = sum_{k<j} relu(0.25 - L1(c_sk, c_sj))^2 / 496
  pull[s]       = mean over ALL B*H*W pixels p of  L1(e_p, c_s,label_p)^2
  loss          = mean_s (push[s] + 0.1 * pull[s])

Key idea: HOST-SORT each core's 73728 pixels by label (order-invariant for
both the segment sums and the pull mean).  Sorted runs are padded to whole
128-pixel tiles (TCB=608 tiles/core, provably enough) so every tile has a
single label.  This removes the one-hot from both launches:

  Launch A (per-tile column sums):
    - embA fp8 [128, 76 groups * 8 tiles * 16 d]; per group g one matmul
      lhsT = embA[:, g] (128 pix x 128 cols), rhs = ones [128, 1] ->
      psum[:, g] = per-(tile,dim) sums over the 128 pixels.  psum [128, 76]
      f32 -> tiny evac + DMA.  Host groups tiles by label -> cents (f64),
      push term on host.

  Launch B (pull term):
    - per tile ONE matmul, contraction rows = 17:
        lhsT (weights) = [embT (16 rows, fp8) ; mask/64 (1 row)]  [17, 128]
        rhs (moving)   = [-I_16 per b (16 rows) ; 64*c_b,lab(t),d (1 row)]
      psum out [pix, (b,d)] = c_b,lab(t),d - e_pix,d  (0 for pad pixels).
    - weights+rhs fused in one fp8 HBM tensor 'wrt' [2 rowgrp, 17, 304, 192]
      (even tiles use PE rows 0:17, odd rows 64:81 -> LDW/MM overlap).
    - 19 psum waves of 32 tiles ([128, 32, 4, 16] f32 = 4 banks, ping-pong).
      Consumer pattern per wave balances the three PSUM/SBUF engines:
        'D' = DVE tensor_reduce(|.|, sum over d) straight from psum
        'A' = ACT Abs -> bf16, Pool h1 halving, DVE h2 + reduce
        'P' = ACT Abs -> bf16, Pool h1+h2+h3 halvings, DVE final halving
    - dist [128, 19, 32, 4] bf16 -> tensor_tensor_reduce (dist^2, accum)
      -> pacc [128, 2] f32 -> host mean.
"""

import os

import numpy as np
import ml_dtypes

import concourse.bass as bass
import concourse.bacc as bacc
import concourse.mybir as mybir
from concourse.tile import TileContext
from concourse.bass_utils import run_bass_kernel_spmd

BF16 = ml_dtypes.bfloat16
FP8 = ml_dtypes.float8_e4m3
F32 = np.float32

# problem constants (hardcoded per contract)
B, H, W, D, K = 4, 384, 384, 16, 32
NCORES = 8
NPIX_TOT = B * H * W              # 589824
NPIX = NPIX_TOT // NCORES         # 73728 per core
P = 128                           # partitions
TCB = 608                         # padded tiles/core: sum_k ceil(cnt_k/128) <= 607
NG_A = TCB // 8                   # 76 matmul groups in launch A
NWAVE = TCB // 32                 # 19 psum waves in launch B
TH4 = TCB // 4                    # 152 tile-quarters per row group
RGB = 32                          # row-group stride (PE 32-row tiles)

PUSH_MARGIN = 0.25
PUSH_W = 1.0
PULL_W = 0.1
NCMP = K * (K - 1) / 2.0

# per-wave consumer kinds for launch B (len NWAVE = 19)
#   D: DVE abs-reduce from psum; A: ACT abs + Pool h1 + DVE h2+reduce;
#   P: ACT abs + Pool h1+h2+h3 + DVE final halving
CONSUMER_PATTERN = os.environ.get("DBG_PATTERN", "DPADPADPADPDAPDPDAD")
USE_TTR = os.environ.get("DBG_TTR", "0") == "1"
# contraction rows: 16 embT + 1 mask row.  NOTE: even/odd tiles run in PE row
# groups 0/2 concurrently; their psum writes MUST land in different banks
# (slot = rg*16 + j//2 below) or the PE hangs (row-tile PSUM bank conflict).
CR = int(os.environ.get("DBG_CR", "17"))

_built = {}


# --------------------------------------------------------------------------
# host-side sort / tiling
# --------------------------------------------------------------------------

def _sort_core(lab):
    """lab [NPIX] int32 -> (order, tile_lab, cnt).

    order: pixel permutation grouping labels; tile_lab [TCB]: label of each
    128-pixel tile (runs padded to whole tiles; leftover tiles labeled 0 and
    fully zero-padded).
    """
    order = np.argsort(lab, kind="stable")
    cnt = np.bincount(lab, minlength=K)
    tile_lab = np.zeros(TCB, dtype=np.int64)
    t = 0
    for k in range(K):
        nt = -(-int(cnt[k]) // P)
        tile_lab[t : t + nt] = k
        t += nt
    return order, tile_lab, cnt, t


def _tiled_emb(emb, lab):
    """emb [NPIX, D] f32, lab [NPIX] -> (tiles [TCB, P, D] f32,
    mask [TCB, P] f32, tile_lab [TCB], cnt [K])."""
    order, tile_lab, cnt, ntl = _sort_core(lab)
    es = emb[order]
    tiles = np.zeros((TCB, P, D), dtype=F32)
    mask = np.zeros((TCB, P), dtype=F32)
    off = 0
    t = 0
    for k in range(K):
        n = int(cnt[k])
        nt = -(-n // P)
        buf = np.zeros((nt * P, D), dtype=F32)
        buf[:n] = es[off : off + n]
        tiles[t : t + nt] = buf.reshape(nt, P, D)
        mbuf = np.zeros(nt * P, dtype=F32)
        mbuf[:n] = 1.0
        mask[t : t + nt] = mbuf.reshape(nt, P)
        off += n
        t += nt
    return tiles, mask, tile_lab, cnt


# --------------------------------------------------------------------------
# launch A: per-tile column sums
# --------------------------------------------------------------------------

def _build_launch_a():
    nc = bacc.Bacc("TRN2", target_bir_lowering=False, debug=False)
    f8 = mybir.dt.float8e4
    bf = mybir.dt.bfloat16
    f32 = mybir.dt.float32

    embA = nc.dram_tensor("embA", [P, TCB * D], f8, kind="ExternalInput")
    outA = nc.dram_tensor("outA", [P, NG_A], f32, kind="ExternalOutput")

    with TileContext(nc) as tc:
        with (
            tc.tile_pool(name="sbuf", bufs=1) as pool,
            tc.tile_pool(name="psum", bufs=1, space="PSUM") as psum_pool,
        ):
            emb_sb = pool.tile([P, TCB * D], f8)
            ones = pool.tile([P, 1], bf)
            evac = pool.tile([P, NG_A], f32)
            psA = psum_pool.tile([P, NG_A], f32)

            nc.vector.memset(ones[:], 1.0)
            NCH = 2
            ch = (TCB * D) // NCH
            for i in range(NCH):
                nc.sync.dma_start(
                    out=emb_sb[:, i * ch : (i + 1) * ch],
                    in_=embA.ap()[:, i * ch : (i + 1) * ch],
                )
            for g in range(NG_A):
                nc.tensor.matmul(
                    psA[:, g : g + 1],
                    emb_sb[:, 128 * g : 128 * g + 128],
                    ones[:],
                    start=True,
                    stop=True,
                )
            nc.vector.tensor_copy(out=evac[:], in_=psA[:])
            nc.sync.dma_start(out=outA.ap(), in_=evac[:])
    nc.compile()
    return nc


def _prep_a(emb_flat, lab_flat):
    in_maps = []
    for c in range(NCORES):
        emb = emb_flat[c * NPIX : (c + 1) * NPIX]
        lab = lab_flat[c * NPIX : (c + 1) * NPIX]
        tiles, _, _, _ = _tiled_emb(emb, lab)
        # embA[p, g, j, d] = tiles[8g + j, p, d]
        embA = np.ascontiguousarray(
            tiles.reshape(NG_A, 8, P, D).transpose(2, 0, 1, 3)
        ).astype(FP8)
        in_maps.append({"embA": embA.reshape(P, TCB * D)})
    return in_maps


def _reduce_a(results, lab_flat):
    """outA [8][P, NG_A] -> cents [B, K, D] float64, counts [B, K]."""
    sums = np.zeros((B, K, D), dtype=np.float64)
    for c in range(NCORES):
        lab = lab_flat[c * NPIX : (c + 1) * NPIX]
        _, tile_lab, _, _ = _sort_core(lab)
        o = results[c]["outA"].astype(np.float64)  # [P=(j,d), g]
        ts = o.reshape(8, D, NG_A).transpose(2, 0, 1).reshape(TCB, D)  # [t, d]
        np.add.at(sums[c // 2], tile_lab, ts)
    cnts = np.zeros((B, K), dtype=np.int64)
    spl = NPIX_TOT // B
    for b in range(B):
        cnts[b] = np.bincount(lab_flat[b * spl : (b + 1) * spl], minlength=K)
    cents = sums / np.maximum(cnts, 1)[:, :, None]
    cents = np.where(cnts[:, :, None] > 0, cents, 0.0)
    return cents, cnts


# --------------------------------------------------------------------------
# launch B: pull term
# --------------------------------------------------------------------------

def _build_launch_b():
    nc = bacc.Bacc("TRN2", target_bir_lowering=False, debug=False)
    f8 = mybir.dt.float8e4
    bf = mybir.dt.bfloat16
    f32 = mybir.dt.float32
    AF = mybir.ActivationFunctionType

    # 4 row groups at partition bases {0,32,64,96}; rows 17:32 of each group
    # are zero pad.  One [128, x] tensor so DMA spreads over all 16 engines.
    wrt = nc.dram_tensor("wrt", [P, TH4 * 192], f8, kind="ExternalInput")
    pacc_d = nc.dram_tensor("pacc", [P, 2], f32, kind="ExternalOutput")

    with TileContext(nc) as tc:
        with (
            tc.tile_pool(name="sbuf", bufs=1) as pool,
            tc.tile_pool(name="work", bufs=3) as wpool,
            tc.tile_pool(name="psum", bufs=2, space="PSUM") as psum_pool,
        ):
            wrt_sb = pool.tile([P, TH4, 192], f8)
            dist = pool.tile([P, NWAVE, 32, 4], bf)
            sqb = pool.tile([P, NWAVE * 32 * 4], bf)
            pacc = pool.tile([P, 2], f32)

            bounds = [0, 40, 96, TH4]
            for i in range(len(bounds) - 1):
                sl = slice(bounds[i], bounds[i + 1])
                nc.sync.dma_start(
                    out=wrt_sb[:, sl, :].rearrange("r t m -> r (t m)"),
                    in_=wrt.ap()[:, bounds[i] * 192 : bounds[i + 1] * 192],
                )

            for w in range(NWAVE):
                ps = psum_pool.tile([P, 32, 4, D], f32, tag="ps", name=f"ps_{w}")
                for j in range(32):
                    t = 32 * w + j
                    rg, th = t % 4, t // 4
                    slot = rg * 8 + j // 4   # row group rg -> psum bank rg
                    nc.tensor.matmul(
                        ps[:, slot, :, :].rearrange("p a b -> p (a b)"),
                        wrt_sb[RGB * rg : RGB * rg + CR, th, 0:128],
                        wrt_sb[RGB * rg : RGB * rg + CR, th, 128:192],
                        start=True,
                        stop=True,
                        tile_position=(RGB * rg, 0),
                    )
                kind = CONSUMER_PATTERN[w]
                with nc.allow_low_precision("dist bf16; error averages out"):
                    if kind == "D":
                        nc.vector.tensor_reduce(
                            out=dist[:, w, :, :],
                            in_=ps[:],
                            axis=mybir.AxisListType.X,
                            op=mybir.AluOpType.add,
                            apply_absolute_value=True,
                        )
                    else:
                        a4 = wpool.tile([P, 32, 4, D], bf, tag="a4")
                        h1 = wpool.tile([P, 32, 4, D // 2], bf, tag="h1")
                        nc.scalar.activation(
                            out=a4[:].rearrange("p a b c -> p (a b c)"),
                            in_=ps[:].rearrange("p a b c -> p (a b c)"),
                            func=AF.Abs,
                        )
                        nc.gpsimd.tensor_tensor(
                            out=h1[:],
                            in0=a4[:, :, :, 0 : D // 2],
                            in1=a4[:, :, :, D // 2 : D],
                            op=mybir.AluOpType.add,
                        )
                        if kind == "A":
                            h2 = wpool.tile([P, 32, 4, D // 4], bf, tag="h2")
                            nc.vector.tensor_tensor(
                                out=h2[:],
                                in0=h1[:, :, :, 0 : D // 4],
                                in1=h1[:, :, :, D // 4 : D // 2],
                                op=mybir.AluOpType.add,
                            )
                            nc.vector.tensor_reduce(
                                out=dist[:, w, :, :],
                                in_=h2[:],
                                axis=mybir.AxisListType.X,
                                op=mybir.AluOpType.add,
                            )
                        else:  # P: pool does h2+h3, DVE final halving add
                            h2 = wpool.tile([P, 32, 4, D // 4], bf, tag="h2")
                            h3 = wpool.tile([P, 32, 4, D // 8], bf, tag="h3")
                            nc.gpsimd.tensor_tensor(
                                out=h2[:],
                                in0=h1[:, :, :, 0 : D // 4],
                                in1=h1[:, :, :, D // 4 : D // 2],
                                op=mybir.AluOpType.add,
                            )
                            nc.gpsimd.tensor_tensor(
                                out=h3[:],
                                in0=h2[:, :, :, 0:2],
                                in1=h2[:, :, :, 2:4],
                                op=mybir.AluOpType.add,
                            )
                            nc.vector.tensor_tensor(
                                out=dist[:, w, :, :],
                                in0=h3[:, :, :, 0],
                                in1=h3[:, :, :, 1],
                                op=mybir.AluOpType.add,
                            )

            # pull accumulation: dist^2 summed, f32 accumulators
            flat = dist[:].rearrange("p a b c -> p (a b c)")
            NTOT = NWAVE * 32 * 4
            hsp = (NWAVE // 2) * 32 * 4
            with nc.allow_low_precision("sq bf16; accum f32"):
                for i, sl in enumerate(
                    (slice(0, hsp), slice(hsp, NTOT))
                ):
                    if USE_TTR:
                        nc.vector.tensor_tensor_reduce(
                            out=sqb[:, sl],
                            in0=flat[:, sl],
                            in1=flat[:, sl],
                            scale=1.0,
                            scalar=0.0,
                            op0=mybir.AluOpType.mult,
                            op1=mybir.AluOpType.add,
                            accum_out=pacc[:, i : i + 1],
                        )
                    else:
                        nc.vector.tensor_tensor(
                            out=sqb[:, sl],
                            in0=flat[:, sl],
                            in1=flat[:, sl],
                            op=mybir.AluOpType.mult,
                        )
                        nc.vector.tensor_reduce(
                            out=pacc[:, i : i + 1],
                            in_=sqb[:, sl],
                            axis=mybir.AxisListType.X,
                            op=mybir.AluOpType.add,
                        )
            nc.sync.dma_start(out=pacc_d.ap(), in_=pacc[:])
    nc.compile()
    return nc


def _prep_b(emb_flat, lab_flat, cents):
    """Fused weights+rhs table 'wrt' per core."""
    cb = cents.astype(F32)  # [B, K, D]
    neg_eye = np.tile(-np.eye(D, dtype=F32), (1, 4))  # [16, 64]
    in_maps = []
    for c in range(NCORES):
        emb = emb_flat[c * NPIX : (c + 1) * NPIX]
        lab = lab_flat[c * NPIX : (c + 1) * NPIX]
        tiles, mask, tile_lab, _ = _tiled_emb(emb, lab)
        # weights [TCB, CR, 128] (rows 17:CR zero-padded)
        w17 = np.zeros((TCB, CR, P), dtype=F32)
        w17[:, 0:D, :] = tiles.transpose(0, 2, 1)
        w17[:, D, :] = mask / 64.0
        # rhs [TCB, CR, 64]
        r17 = np.zeros((TCB, CR, 4 * D), dtype=F32)
        r17[:, 0:D, :] = neg_eye[None]
        crow = cb[:, tile_lab, :].transpose(1, 0, 2).reshape(TCB, 4 * D)
        r17[:, D, :] = 64.0 * crow
        fused = np.concatenate([w17, r17], axis=2)  # [TCB, CR, 192]
        wrt = np.zeros((4, RGB, TH4, 192), dtype=FP8)
        for g in range(4):
            wrt[g, :CR] = fused[g::4].transpose(1, 0, 2).astype(FP8)
        in_maps.append({"wrt": wrt.reshape(P, TH4 * 192)})
    return in_maps


def _push_host(cents):
    """push[b] = sum_{k<j} relu(m - L1(c_k, c_j))^2 / 496 on host (tiny)."""
    cb = cents.astype(np.float64)  # [B, K, D]
    d = np.abs(cb[:, :, None, :] - cb[:, None, :, :]).sum(axis=-1)  # [B, K, K]
    m = np.maximum(PUSH_MARGIN - d, 0.0)
    iu = np.triu(np.ones((K, K), dtype=bool), k=1)
    return (m * m * iu[None]).sum(axis=(1, 2)) / NCMP  # [B]


def _get(name):
    if name not in _built:
        if name == "A":
            _built[name] = _build_launch_a()
        else:
            _built[name] = _build_launch_b()
    return _built[name]


def run_launches(embeddings, labels, trace=False, trace_kwargs=None):
    """Returns (loss_scalar, resA, resB) — resA/resB are BassKernelResults."""
    emb_flat = np.ascontiguousarray(np.asarray(embeddings), dtype=F32).reshape(
        NPIX_TOT, D
    )
    lab_flat = np.ascontiguousarray(np.asarray(labels), dtype=np.int32).reshape(
        NPIX_TOT
    )
    core_ids = list(range(NCORES))

    kwA = dict(trace=trace, **(trace_kwargs or {}))
    resA = run_bass_kernel_spmd(_get("A"), _prep_a(emb_flat, lab_flat), core_ids, **kwA)
    cents, _ = _reduce_a(resA.results, lab_flat)

    resB = run_bass_kernel_spmd(
        _get("B"), _prep_b(emb_flat, lab_flat, cents), core_ids, **kwA
    )
    pull_sum = 0.0
    for c in range(NCORES):
        pull_sum += float(resB.results[c]["pacc"].astype(np.float64).sum())
    pull_mean = pull_sum / NPIX_TOT / B  # mean over b of pull[b]

    push = _push_host(cents)
    loss = np.mean(PUSH_W * push) + PULL_W * pull_mean
    return np.array(loss, dtype=F32), resA, resB


def kernel(embeddings, labels):
    loss, _, _ = run_launches(embeddings, labels, trace=False)
    return loss
